# revision 4
# baseline (speedup 1.0000x reference)
"""Bass/Tile kernel for nn_PostProModel on 8 Trainium2 NeuronCores (V2).

Per batch element b (65536 total, data-parallel over 8 cores):
  x      = [prob[b] | member_idx]          [16, 4]
  hidden = relu(x @ W1 + b1)               [16, 128]
  S      = hidden @ hidden^T               [16, 16]
  A      = softmax(S, axis=-1)
  out    = (A @ hidden) @ W2 + b2          [16, 3]

V2 design (per core: 8192 elements = 131072 (b,m) columns, 256 strips of 512):
  - hidden scaled by SC=sqrt(128*log2(e)) folded into W1/b1 so the gram
    psum directly holds S' = S*128*log2(e).
  - strips processed in PAIRS: one hidden matmul (N=1024) + one relu
    instruction per pair (pair-batched to amortize engine overhead).
  - gram+GO merged: rhs = [hidden_block(128) | w2z(4)] per pass (N=132),
    one matmul + one weight load per pass.
  - softmax exp via "Schraudolph bf16": ONE vector tensor_tensor per strip
    computes int16(S' + bmask) whose bits reinterpret as bf16 ~= exp(S)
    with the block-diagonal mask folded in as an additive bias. No ACT
    exp, no separate mask multiply.
  - out matmul: lhsT = E (bf16 view of the int16 tile), rhs = Gpp =
    [GO | ones-col], accumulating output AND softmax denominator.
  - division by the denominator and the +b2 bias happen on the HOST.
"""

import sys
import numpy as np

sys.path.insert(0, "/opt/trn_rl_repo")

N_CORES = 8
B_TOTAL, M, C, H = 65536, 16, 3, 128
B_CORE = B_TOTAL // N_CORES            # 8192
COLS = B_CORE * M                      # 131072
NB = 512                               # columns per strip
NPASS = 4                              # 128-col passes per strip
NSTRIP = COLS // NB                    # 256
NPAIR = NSTRIP // 2                    # 128
CHUNK = 8                              # strips per input DMA
OUTG = 8                               # strips per output DMA

SCL2 = 128.0 * np.log2(np.e)           # S' = S * SCL2
SC = float(np.sqrt(SCL2))              # folded into W1, b1; 1/SC into W2
MAGIC = 16256.0 - 8.0 + 0.5            # 127*128 - C + round-bias
MASKOFF = 16000.0                      # pushed into denormal-land when masked

_CACHE = {}


def _build(nstrip, padk):
    import concourse.bacc as bacc
    import concourse.tile as tile
    from concourse import mybir

    f32 = mybir.dt.float32
    i16 = mybir.dt.int16
    DT = mybir.dt.bfloat16
    Alu = mybir.AluOpType
    Act = mybir.ActivationFunctionType

    npair = nstrip // 2
    KROWS = 128 if padk else 4

    nc = bacc.Bacc("TRN2")
    probT = nc.dram_tensor("probT", [4, COLS], DT, kind="ExternalInput")
    w1 = nc.dram_tensor("w1", [KROWS, H], DT, kind="ExternalInput")
    b1 = nc.dram_tensor("b1", [H, 1], f32, kind="ExternalInput")
    w2z8 = nc.dram_tensor("w2z8", [H, 32], DT, kind="ExternalInput")
    bmask = nc.dram_tensor("bmask", [128, 128], f32, kind="ExternalInput")
    outb = nc.dram_tensor("outb", [nstrip // OUTG, 128, 16 * OUTG], f32,
                          kind="ExternalOutput")

    # engine load balancing (ns-ish costs; ACT @1.2GHz ovh 352, DVE @0.96 ovh 120)
    load = {"A": 0.0, "D": 0.0}

    def pick(cost_a, cost_d):
        if load["A"] + cost_a <= load["D"] + cost_d:
            load["A"] += cost_a
            return "A"
        load["D"] += cost_d
        return "D"

    with tile.TileContext(nc) as tc:
        from contextlib import ExitStack
        with ExitStack() as ctx:
            singles = ctx.enter_context(tc.tile_pool(name="singles", bufs=1))
            px = ctx.enter_context(tc.tile_pool(name="px", bufs=3))
            pe = ctx.enter_context(tc.tile_pool(name="pe", bufs=3))
            pout = ctx.enter_context(tc.tile_pool(name="pout", bufs=2))
            pH = ctx.enter_context(tc.tile_pool(name="pH", bufs=2, space="PSUM"))
            pS = ctx.enter_context(tc.tile_pool(name="pS", bufs=2, space="PSUM"))

            w1_t = singles.tile([KROWS, H], DT)
            nc.sync.dma_start(out=w1_t, in_=w1[:, :])
            b1_t = singles.tile([H, 1], f32)
            nc.sync.dma_start(out=b1_t, in_=b1[:, :])
            bmask_t = singles.tile([128, 128], f32)
            nc.sync.dma_start(out=bmask_t, in_=bmask[:, :])

            # hT ring: pair tiles [128, 8*132]; w2z pre-filled at cols 132k+128
            hTs = []
            for i in range(3):
                t = singles.tile([128, 8 * 132], DT, tag=f"hT{i}")
                tv = t[:, :].rearrange("q (g c) -> q g c", g=8)
                nc.sync.dma_start(out=tv[:, :, 128:132], in_=w2z8[:, :])
                hTs.append(t)

            # Gpp ring: [128, 16] bf16, col3 of each 4-block preset to 1.0
            gpps = []
            for i in range(3):
                t = singles.tile([128, 16], DT, tag=f"gpp{i}")
                nc.vector.memset(t[:, :], 1.0)
                gpps.append(t)

            # probT chunk tiles (K-padded when padk)
            xts = []
            if padk:
                for i in range(3):
                    t = singles.tile([128, NB * CHUNK], DT, tag=f"xt{i}")
                    nc.vector.memset(t[:, :], 0.0)
                    xts.append(t)

            def st_dma(c):
                if padk:
                    xT = xts[c % 3]
                    nc.sync.dma_start(out=xT[0:4, :],
                                      in_=probT[:, NB * CHUNK * c:
                                                NB * CHUNK * (c + 1)])
                else:
                    xT = px.tile([4, NB * CHUNK], DT, tag="xT")
                    nc.sync.dma_start(out=xT,
                                      in_=probT[:, NB * CHUNK * c:
                                                NB * CHUNK * (c + 1)])
                return xT

            def st_hidden(g, xT):
                off = (g % (CHUNK // 2)) * 2 * NB
                psumH = pH.tile([128, 2 * NB], f32, tag="psumH")
                nc.tensor.matmul(psumH[:, 0:NB], w1_t[:, :],
                                 xT[:, off:off + NB], start=True, stop=True)
                nc.tensor.matmul(psumH[:, NB:2 * NB], w1_t[:, :],
                                 xT[:, off + NB:off + 2 * NB],
                                 start=True, stop=True)
                hT = hTs[g % 3]
                hv = hT[:, :].rearrange("q (g c) -> q g c", g=8)[:, :, 0:128]
                e = pick(1147.0, 1192.0)
                if e == "A":
                    nc.scalar.activation(hv, psumH[:, :], Act.Relu,
                                         bias=b1_t[:, 0:1], scale=1.0)
                else:
                    pv = psumH[:, :].rearrange("q (g c) -> q g c", g=8)
                    nc.vector.tensor_scalar(hv, pv, scalar1=b1_t[:, 0:1],
                                            scalar2=0.0, op0=Alu.add,
                                            op1=Alu.max)
                return hT, psumH

            def st_gram(g, t, hT):
                # strip s = 2g + t ; hT blocks 4t..4t+3
                psumS = pS.tile([128, 1024], f32, tag="psumS")
                for p in range(NPASS):
                    base = 132 * (4 * t + p)
                    nc.tensor.matmul(psumS[:, 256 * p:256 * p + 132],
                                     hT[:, base:base + 128],
                                     hT[:, base:base + 132],
                                     start=True, stop=True)
                return psumS

            def st_soft(s, psumS):
                Ei = pe.tile([128, 512], i16, tag="Ei")
                ev = Ei[:, :].rearrange("q (g c) -> q g c", g=NPASS)
                sv = psumS[:, :].rearrange("q (g c) -> q g c", g=NPASS)
                nc.vector.tensor_tensor(
                    ev, sv[:, :, 0:128],
                    bmask_t[:, None, :].broadcast_to([128, NPASS, 128]),
                    op=Alu.add)
                load["D"] += 658.0
                # Gpp: copy GO cols (128:131 of each 256-block) -> cols 0:3
                gpp = gpps[s % 3]
                gv = gpp[:, :].rearrange("q (g c) -> q g c", g=NPASS)
                e = pick(303.0, 138.0)
                if e == "A":
                    nc.scalar.copy(gv[:, :, 0:3], sv[:, :, 128:131])
                else:
                    nc.vector.tensor_copy(gv[:, :, 0:3], sv[:, :, 128:131])
                return Ei, gpp

            def st_out(g, t, Ei, gpp, psumS, psumH):
                Eb = Ei[:, :].bitcast(mybir.dt.bfloat16)
                for p in range(NPASS):
                    nc.tensor.matmul(psumH[:, 16 * t + 4 * p:16 * t + 4 * p + 4],
                                     Eb[:, 128 * p:128 * p + 128],
                                     gpp[:, 4 * p:4 * p + 4],
                                     start=True, stop=True)

            def st_ocopy(g, psumH, outS):
                # both strips of pair g: psumH cols 0:32 -> staging
                gg = g % (OUTG // 2)
                e = pick(347.0, 158.0)
                if e == "A":
                    nc.scalar.copy(outS[:, 32 * gg:32 * gg + 32],
                                   psumH[:, 0:32])
                else:
                    nc.vector.tensor_copy(outS[:, 32 * gg:32 * gg + 32],
                                          psumH[:, 0:32])

            # software pipeline over pairs
            live = {}

            def stage_a(g):
                # dma + hidden + relu for pair g
                if g % (CHUNK // 2) == 0:
                    live[f"x{g // (CHUNK // 2)}"] = st_dma(g // (CHUNK // 2))
                xT = live[f"x{g // (CHUNK // 2)}"]
                live[g] = st_hidden(g, xT)

            outS_box = {}
            stage_a(0)
            for g in range(npair):
                if g + 1 < npair:
                    stage_a(g + 1)
                hT, psumH = live.pop(g)
                if g % (OUTG // 2) == 0:
                    outS = pout.tile([128, 16 * OUTG], f32, tag="outS")
                    outS_box["t"] = outS
                outS = outS_box["t"]
                parts = []
                for t in range(2):
                    s = 2 * g + t
                    psumS = st_gram(g, t, hT)
                    Ei, gpp = st_soft(s, psumS)
                    parts.append((Ei, gpp, psumS))
                for t in range(2):
                    Ei, gpp, psumS = parts[t]
                    st_out(g, t, Ei, gpp, psumS, psumH)
                st_ocopy(g, psumH, outS)
                if (g + 1) % (OUTG // 2) == 0:
                    u = g // (OUTG // 2)
                    nc.sync.dma_start(out=outb[u, :, :], in_=outS[:, :])

    nc.finalize()
    return nc


def _prep_core_inputs(prob_core, W1, b1, W2, padk):
    import ml_dtypes
    bf16 = ml_dtypes.bfloat16
    pT = np.ascontiguousarray(prob_core.reshape(-1, C).T)        # [3, COLS]
    idx = np.tile(np.arange(M, dtype=np.float32), B_CORE)[None]  # [1, COLS]
    probT_aug = np.ascontiguousarray(np.concatenate([pT, idx], axis=0))

    W1s = (np.asarray(W1, np.float32) * SC)
    if padk:
        w1z = np.zeros((128, H), np.float32)
        w1z[0:4] = W1s
    else:
        w1z = W1s
    b1s = np.asarray(b1, np.float32).reshape(H, 1) * SC
    w2s = np.asarray(W2, np.float32) / SC                        # [H, 3]
    w2z = np.concatenate([w2s, np.zeros((H, 1), np.float32)], axis=1)
    w2z8 = np.tile(w2z, (1, 8))                                  # [H, 32]

    mask = np.kron(np.eye(8, dtype=np.float32),
                   np.ones((16, 16), np.float32))
    bmask = (MAGIC - MASKOFF * (1.0 - mask)).astype(np.float32)

    return {
        "probT": probT_aug.astype(bf16),
        "w1": np.ascontiguousarray(w1z).astype(bf16),
        "b1": np.ascontiguousarray(b1s),
        "w2z8": np.ascontiguousarray(w2z8).astype(bf16),
        "bmask": np.ascontiguousarray(bmask),
    }


def _postprocess(outb_arr, b2):
    # outb [32, 128, 128]: [chunk, q=(e,m), 32*gg + 16*t + 4p + cc]
    # strip s = 8*chunk + 2*gg + t ; b = 32*s + 8*p + e
    nch = outb_arr.shape[0]
    r = outb_arr.reshape(nch, 8, 16, OUTG // 2, 2, 4, 4)  # ch, e, m, gg, t, p, cc
    r = r.transpose(0, 3, 4, 5, 1, 2, 6)                  # ch, gg, t, p, e, m, cc
    r = r.reshape(-1, M, 4)
    out = r[..., 0:3] / r[..., 3:4] + np.asarray(b2, np.float32)[None, None, :]
    return np.ascontiguousarray(out)


def _maybe_patch_ldwopt():
    import os
    if os.environ.get("PPK_LDWOPT", "0") != "1":
        return
    import concourse.bass_utils as bu
    if getattr(bu, "_ppk_ldw_patched", False):
        return
    orig = bu.run_command

    def patched(argv, **kw):
        argv = [a.replace("--enable-ldw-opt=false", "--enable-ldw-opt=true")
                if isinstance(a, str) else a for a in argv]
        return orig(argv, **kw)

    bu.run_command = patched
    bu._ppk_ldw_patched = True


def kernel(prob, W1, b1, W2, b2, _trace=False):
    import os
    from concourse.bass_utils import run_bass_kernel_spmd
    _maybe_patch_ldwopt()
    padk = os.environ.get("PPK_LDWOPT", "0") == "1"

    if "nc" not in _CACHE:
        _CACHE["nc"] = _build(NSTRIP, padk)
    nc = _CACHE["nc"]

    prob = np.asarray(prob, np.float32)
    in_maps = []
    for ci in range(N_CORES):
        pc = prob[ci * B_CORE:(ci + 1) * B_CORE]
        in_maps.append(_prep_core_inputs(pc, W1, b1, W2, padk))
    res = run_bass_kernel_spmd(nc, in_maps, list(range(N_CORES)),
                               trace=_trace)
    _CACHE["last_result"] = res
    out = np.zeros((B_TOTAL, M, C), np.float32)
    for ci in range(N_CORES):
        o = _postprocess(res.results[ci]["outb"], b2)
        out[ci * B_CORE:ci * B_CORE + o.shape[0]] = o
    return out


# revision 5
# speedup vs baseline: 1.0011x; 1.0011x over previous
"""Bass/Tile kernel for nn_PostProModel on 8 Trainium2 NeuronCores (V2).

Per batch element b (65536 total, data-parallel over 8 cores):
  x      = [prob[b] | member_idx]          [16, 4]
  hidden = relu(x @ W1 + b1)               [16, 128]
  S      = hidden @ hidden^T               [16, 16]
  A      = softmax(S, axis=-1)
  out    = (A @ hidden) @ W2 + b2          [16, 3]

V2 design (per core: 8192 elements = 131072 (b,m) columns, 256 strips of 512):
  - hidden scaled by SC=sqrt(128*log2(e)) folded into W1/b1 so the gram
    psum directly holds S' = S*128*log2(e).
  - strips processed in PAIRS: one hidden matmul (N=1024) + one relu
    instruction per pair (pair-batched to amortize engine overhead).
  - gram+GO merged: rhs = [hidden_block(128) | w2z(4)] per pass (N=132),
    one matmul + one weight load per pass.
  - softmax exp via "Schraudolph bf16": ONE vector tensor_tensor per strip
    computes int16(S' + bmask) whose bits reinterpret as bf16 ~= exp(S)
    with the block-diagonal mask folded in as an additive bias. No ACT
    exp, no separate mask multiply.
  - out matmul: lhsT = E (bf16 view of the int16 tile), rhs = Gpp =
    [GO | ones-col], accumulating output AND softmax denominator.
  - division by the denominator and the +b2 bias happen on the HOST.
"""

import sys
import numpy as np

sys.path.insert(0, "/opt/trn_rl_repo")

N_CORES = 8
B_TOTAL, M, C, H = 65536, 16, 3, 128
B_CORE = B_TOTAL // N_CORES            # 8192
COLS = B_CORE * M                      # 131072
NB = 512                               # columns per strip
NPASS = 4                              # 128-col passes per strip
NSTRIP = COLS // NB                    # 256
NPAIR = NSTRIP // 2                    # 128
CHUNK = 8                              # strips per input DMA
OUTG = 8                               # strips per output DMA

SCL2 = 128.0 * np.log2(np.e)           # S' = S * SCL2
SC = float(np.sqrt(SCL2))              # folded into W1, b1; 1/SC into W2
MAGIC = 16256.0 - 8.0 + 0.5            # 127*128 - C + round-bias
MASKOFF = 16000.0                      # pushed into denormal-land when masked

_CACHE = {}


def _build(nstrip, padk):
    import concourse.bacc as bacc
    import concourse.tile as tile
    from concourse import mybir

    f32 = mybir.dt.float32
    i16 = mybir.dt.int16
    DT = mybir.dt.bfloat16
    Alu = mybir.AluOpType
    Act = mybir.ActivationFunctionType

    npair = nstrip // 2
    KROWS = 128 if padk else 4

    nc = bacc.Bacc("TRN2")
    probT = nc.dram_tensor("probT", [4, COLS], DT, kind="ExternalInput")
    w1 = nc.dram_tensor("w1", [KROWS, H], DT, kind="ExternalInput")
    b1 = nc.dram_tensor("b1", [H, 1], f32, kind="ExternalInput")
    w2z8 = nc.dram_tensor("w2z8", [H, 32], DT, kind="ExternalInput")
    bmask = nc.dram_tensor("bmask", [128, 128], f32, kind="ExternalInput")
    outb = nc.dram_tensor("outb", [nstrip // OUTG, 128, 16 * OUTG], f32,
                          kind="ExternalOutput")

    # engine load balancing (ns-ish costs; ACT @1.2GHz ovh 352, DVE @0.96 ovh 120)
    load = {"A": 0.0, "D": 0.0}

    def pick(cost_a, cost_d):
        if load["A"] + cost_a <= load["D"] + cost_d:
            load["A"] += cost_a
            return "A"
        load["D"] += cost_d
        return "D"

    with tile.TileContext(nc) as tc:
        from contextlib import ExitStack
        with ExitStack() as ctx:
            singles = ctx.enter_context(tc.tile_pool(name="singles", bufs=1))
            px = ctx.enter_context(tc.tile_pool(name="px", bufs=3))
            pe = ctx.enter_context(tc.tile_pool(name="pe", bufs=3))
            pout = ctx.enter_context(tc.tile_pool(name="pout", bufs=2))
            pH = ctx.enter_context(tc.tile_pool(name="pH", bufs=2, space="PSUM"))
            pS = ctx.enter_context(tc.tile_pool(name="pS", bufs=2, space="PSUM"))

            w1_t = singles.tile([KROWS, H], DT)
            nc.sync.dma_start(out=w1_t, in_=w1[:, :])
            b1_t = singles.tile([H, 1], f32)
            nc.sync.dma_start(out=b1_t, in_=b1[:, :])
            bmask_t = singles.tile([128, 128], f32)
            nc.sync.dma_start(out=bmask_t, in_=bmask[:, :])

            # hT ring: pair tiles [128, 8*132]; w2z pre-filled at cols 132k+128
            hTs = []
            for i in range(3):
                t = singles.tile([128, 8 * 132], DT, tag=f"hT{i}")
                tv = t[:, :].rearrange("q (g c) -> q g c", g=8)
                nc.sync.dma_start(out=tv[:, :, 128:132], in_=w2z8[:, :])
                hTs.append(t)

            # Gpp ring: [128, 16] bf16, col3 of each 4-block preset to 1.0
            gpps = []
            for i in range(3):
                t = singles.tile([128, 16], DT, tag=f"gpp{i}")
                nc.vector.memset(t[:, :], 1.0)
                gpps.append(t)

            # probT chunk tiles (K-padded when padk)
            xts = []
            if padk:
                for i in range(3):
                    t = singles.tile([128, NB * CHUNK], DT, tag=f"xt{i}")
                    nc.vector.memset(t[:, :], 0.0)
                    xts.append(t)

            def st_dma(c):
                if padk:
                    xT = xts[c % 3]
                    nc.sync.dma_start(out=xT[0:4, :],
                                      in_=probT[:, NB * CHUNK * c:
                                                NB * CHUNK * (c + 1)])
                else:
                    xT = px.tile([4, NB * CHUNK], DT, tag="xT")
                    nc.sync.dma_start(out=xT,
                                      in_=probT[:, NB * CHUNK * c:
                                                NB * CHUNK * (c + 1)])
                return xT

            def st_hidden(g, xT):
                off = (g % (CHUNK // 2)) * 2 * NB
                psumH = pH.tile([128, 2 * NB], f32, tag="psumH")
                nc.tensor.matmul(psumH[:, 0:NB], w1_t[:, :],
                                 xT[:, off:off + NB], start=True, stop=True)
                nc.tensor.matmul(psumH[:, NB:2 * NB], w1_t[:, :],
                                 xT[:, off + NB:off + 2 * NB],
                                 start=True, stop=True)
                hT = hTs[g % 3]
                hv = hT[:, :].rearrange("q (g c) -> q g c", g=8)[:, :, 0:128]
                e = pick(1147.0, 1192.0)
                if e == "A":
                    nc.scalar.activation(hv, psumH[:, :], Act.Relu,
                                         bias=b1_t[:, 0:1], scale=1.0)
                else:
                    pv = psumH[:, :].rearrange("q (g c) -> q g c", g=8)
                    nc.vector.tensor_scalar(hv, pv, scalar1=b1_t[:, 0:1],
                                            scalar2=0.0, op0=Alu.add,
                                            op1=Alu.max)
                return hT, psumH

            def st_gram(g, t, hT):
                # strip s = 2g + t ; hT blocks 4t..4t+3
                psumS = pS.tile([128, 1024], f32, tag="psumS")
                for p in range(NPASS):
                    base = 132 * (4 * t + p)
                    nc.tensor.matmul(psumS[:, 256 * p:256 * p + 132],
                                     hT[:, base:base + 128],
                                     hT[:, base:base + 132],
                                     start=True, stop=True)
                return psumS

            def st_soft(s, psumS):
                Ei = pe.tile([128, 512], i16, tag="Ei")
                ev = Ei[:, :].rearrange("q (g c) -> q g c", g=NPASS)
                sv = psumS[:, :].rearrange("q (g c) -> q g c", g=NPASS)
                nc.vector.tensor_tensor(
                    ev, sv[:, :, 0:128],
                    bmask_t[:, None, :].broadcast_to([128, NPASS, 128]),
                    op=Alu.add)
                load["D"] += 658.0
                # Gpp: copy GO cols (128:131 of each 256-block) -> cols 0:3
                gpp = gpps[s % 3]
                gv = gpp[:, :].rearrange("q (g c) -> q g c", g=NPASS)
                e = pick(303.0, 138.0)
                if e == "A":
                    nc.scalar.copy(gv[:, :, 0:3], sv[:, :, 128:131])
                else:
                    nc.vector.tensor_copy(gv[:, :, 0:3], sv[:, :, 128:131])
                return Ei, gpp

            def st_out(g, t, Ei, gpp, psumS, psumH):
                Eb = Ei[:, :].bitcast(mybir.dt.bfloat16)
                for p in range(NPASS):
                    nc.tensor.matmul(psumH[:, 16 * t + 4 * p:16 * t + 4 * p + 4],
                                     Eb[:, 128 * p:128 * p + 128],
                                     gpp[:, 4 * p:4 * p + 4],
                                     start=True, stop=True)

            def st_ocopy(g, psumH, outS):
                # both strips of pair g: psumH cols 0:32 -> staging
                gg = g % (OUTG // 2)
                e = pick(347.0, 158.0)
                if e == "A":
                    nc.scalar.copy(outS[:, 32 * gg:32 * gg + 32],
                                   psumH[:, 0:32])
                else:
                    nc.vector.tensor_copy(outS[:, 32 * gg:32 * gg + 32],
                                          psumH[:, 0:32])

            # software pipeline over pairs, depth 2:
            # iter g:  out/ocopy(g-1) | dma lookahead | hidden+relu(g+1) |
            #          gram+schr+gpp(g)
            live = {}
            chunks = {}

            def st_dma_maybe(c):
                if c * CHUNK // 2 < npair and c not in chunks:
                    chunks[c] = st_dma(c)

            def stage_a(g):
                xT = chunks[g // (CHUNK // 2)]
                live[g] = st_hidden(g, xT)

            outS_box = {}

            def stage_out(g):
                # out-MMs + ocopy for pair g (after pair g+1's grams emitted)
                hT, psumH, parts = live.pop(g)
                if g % (OUTG // 2) == 0:
                    outS = pout.tile([128, 16 * OUTG], f32, tag="outS")
                    outS_box["t"] = outS
                outS = outS_box["t"]
                for t in range(2):
                    Ei, gpp, psumS = parts[t]
                    st_out(g, t, Ei, gpp, psumS, psumH)
                st_ocopy(g, psumH, outS)
                if (g + 1) % (OUTG // 2) == 0:
                    u = g // (OUTG // 2)
                    nc.sync.dma_start(out=outb[u, :, :], in_=outS[:, :])

            st_dma_maybe(0)
            st_dma_maybe(1)
            stage_a(0)
            for g in range(npair):
                if g >= 2:
                    stage_out(g - 1)
                st_dma_maybe(g // (CHUNK // 2) + 2)
                if g + 1 < npair:
                    stage_a(g + 1)
                hT, psumH = live[g]
                parts = []
                for t in range(2):
                    s = 2 * g + t
                    psumS = st_gram(g, t, hT)
                    Ei, gpp = st_soft(s, psumS)
                    parts.append((Ei, gpp, psumS))
                live[g] = (hT, psumH, parts)
                if g == 0:
                    stage_out(0)
            stage_out(npair - 1)

    nc.finalize()
    return nc


def _prep_core_inputs(prob_core, W1, b1, W2, padk):
    import ml_dtypes
    bf16 = ml_dtypes.bfloat16
    pT = np.ascontiguousarray(prob_core.reshape(-1, C).T)        # [3, COLS]
    idx = np.tile(np.arange(M, dtype=np.float32), B_CORE)[None]  # [1, COLS]
    probT_aug = np.ascontiguousarray(np.concatenate([pT, idx], axis=0))

    W1s = (np.asarray(W1, np.float32) * SC)
    if padk:
        w1z = np.zeros((128, H), np.float32)
        w1z[0:4] = W1s
    else:
        w1z = W1s
    b1s = np.asarray(b1, np.float32).reshape(H, 1) * SC
    w2s = np.asarray(W2, np.float32) / SC                        # [H, 3]
    w2z = np.concatenate([w2s, np.zeros((H, 1), np.float32)], axis=1)
    w2z8 = np.tile(w2z, (1, 8))                                  # [H, 32]

    mask = np.kron(np.eye(8, dtype=np.float32),
                   np.ones((16, 16), np.float32))
    bmask = (MAGIC - MASKOFF * (1.0 - mask)).astype(np.float32)

    return {
        "probT": probT_aug.astype(bf16),
        "w1": np.ascontiguousarray(w1z).astype(bf16),
        "b1": np.ascontiguousarray(b1s),
        "w2z8": np.ascontiguousarray(w2z8).astype(bf16),
        "bmask": np.ascontiguousarray(bmask),
    }


def _postprocess(outb_arr, b2):
    # outb [32, 128, 128]: [chunk, q=(e,m), 32*gg + 16*t + 4p + cc]
    # strip s = 8*chunk + 2*gg + t ; b = 32*s + 8*p + e
    nch = outb_arr.shape[0]
    r = outb_arr.reshape(nch, 8, 16, OUTG // 2, 2, 4, 4)  # ch, e, m, gg, t, p, cc
    r = r.transpose(0, 3, 4, 5, 1, 2, 6)                  # ch, gg, t, p, e, m, cc
    r = r.reshape(-1, M, 4)
    out = r[..., 0:3] / r[..., 3:4] + np.asarray(b2, np.float32)[None, None, :]
    return np.ascontiguousarray(out)


def _maybe_patch_ldwopt():
    import os
    if os.environ.get("PPK_LDWOPT", "0") != "1":
        return
    import concourse.bass_utils as bu
    if getattr(bu, "_ppk_ldw_patched", False):
        return
    orig = bu.run_command

    def patched(argv, **kw):
        argv = [a.replace("--enable-ldw-opt=false", "--enable-ldw-opt=true")
                if isinstance(a, str) else a for a in argv]
        return orig(argv, **kw)

    bu.run_command = patched
    bu._ppk_ldw_patched = True


def kernel(prob, W1, b1, W2, b2, _trace=False):
    import os
    from concourse.bass_utils import run_bass_kernel_spmd
    _maybe_patch_ldwopt()
    padk = os.environ.get("PPK_LDWOPT", "0") == "1"

    if "nc" not in _CACHE:
        _CACHE["nc"] = _build(NSTRIP, padk)
    nc = _CACHE["nc"]

    prob = np.asarray(prob, np.float32)
    in_maps = []
    for ci in range(N_CORES):
        pc = prob[ci * B_CORE:(ci + 1) * B_CORE]
        in_maps.append(_prep_core_inputs(pc, W1, b1, W2, padk))
    res = run_bass_kernel_spmd(nc, in_maps, list(range(N_CORES)),
                               trace=_trace)
    _CACHE["last_result"] = res
    out = np.zeros((B_TOTAL, M, C), np.float32)
    for ci in range(N_CORES):
        o = _postprocess(res.results[ci]["outb"], b2)
        out[ci * B_CORE:ci * B_CORE + o.shape[0]] = o
    return out
